# revision 7
# baseline (speedup 1.0000x reference)
"""Trainium2 Bass kernel for nn_BehaviorVelocity (velocity-driven swap sim + smoothing).

Sharding: data-parallel over batch B=16 across 8 cores (2 images/core, no collectives).

Layout per 512x512 image: partition p holds rows 4p..4p+3 as free-dim "slots".
Padded field = [128, 6 slots, 514 cols]:
  slot 0 = row 4p-1 (y-halo lo), slots 1..4 = rows 4p..4p+3, slot 5 = row 4p+4 (y-halo hi)
  col 0 = x=511 (wrap), cols 1..512 = x=0..511, col 513 = x=0 (wrap)
y-halos: partition-shift SBUF DMAs (+1-row torus wrap DMA). x-halos: tiny strided copies.
All spatial shifts then become free-dim AP offsets (compute ops must start at partition 0).

Channels 3,4 (vy,vx) stay f32. Channels (1,2),(5,6),(7,0) are packed as bf16 pairs
inside int32 containers so each (mode-less, 1x) copy_predicated moves 2 channels.

Sector selection replicates floor(8*arccos-angle+0.5) via threshold compares in the
squared domain:  vx <= K*(mag+0.001)  <=>  (vx<=0) or (vx^2 <= K^2*magp2)  with
magp2 = m2 + 0.002*mag + 1e-6, so the ACT-sqrt LUT error only enters the tiny
0.002*mag term (~5e-8 boundary shift instead of ~3e-5).
"""

import sys

sys.path.insert(0, "/opt/trn_rl_repo")

import numpy as np

import concourse.bacc as bacc
import concourse.mybir as mybir
from concourse.tile import TileContext
from concourse.bass_utils import run_bass_kernel_spmd

dt = mybir.dt
Alu = mybir.AluOpType
Act = mybir.ActivationFunctionType

P = 128          # partitions
S = 4            # row-slots per partition (512 rows / 128)
W = 512
Wp = W + 2       # 514 with x-halo cols
NB = 2           # batch images per core
NCORES = 8

_DY = [0, 1, 1, 1, 0, -1, -1, -1]
_DX = [1, 1, 0, -1, -1, -1, 0, 1]

K0SQ = float(np.cos(np.pi / 8) ** 2)      # 0.85355339059
K1SQ = float(np.cos(3 * np.pi / 8) ** 2)  # 0.14644660941

PACKS = [(1, 2), (5, 6), (7, 0)]  # (lo, hi) bf16 pair in int32; E-channel 0 = hi of last

USE_BCAST_CVALS = True

_cache = {}


def _interior(t):
    return t[:, 1:1 + S, 1:1 + W]


def _view(t, dy, dx):
    # value of neighbor at (y+dy, x+dx) for each interior pixel
    return t[:, 1 + dy:1 + S + dy, 1 + dx:1 + W + dx]


def _bf16_views(t_i32):
    """(lo, hi) bf16 strided views [P,6,Wp] of an int32 [P,6,Wp] padded tile."""
    b = t_i32[:].bitcast(dt.bfloat16)            # [P, 6, 2*Wp]
    b = b.rearrange("p s (c two) -> p s c two", two=2)
    return b[:, :, :, 0], b[:, :, :, 1]


class _Emit:
    def __init__(self, nk):
        self.nk = nk  # 3x3 conv kernel (already /18)
        nc = self.nc = bacc.Bacc()
        self.win = nc.declare_dram_parameter("w", [NB, 8, 512, 512], dt.float32, isOutput=False)
        self.wout = nc.declare_dram_parameter("o", [NB, 8, 512, 512], dt.float32, isOutput=True)

    def build(self):
        nc = self.nc
        with TileContext(nc) as tc:
            self.tc = tc
            with (
                tc.tile_pool(name="pconst", bufs=1) as pconst,
                tc.tile_pool(name="pw32", bufs=5) as pw32,      # f32 padded [P,6,Wp]
                tc.tile_pool(name="ppack", bufs=4) as ppack,    # int32 padded [P,6,Wp]
                tc.tile_pool(name="pmask", bufs=9) as pmask,    # u8 tight [P,S,W]: Wm2 + equ8
                tc.tile_pool(name="pf32t", bufs=4) as pf32t,    # f32 tight [P,S,W]
                tc.tile_pool(name="pu8t", bufs=8) as pu8t,      # u8 tight: phase-A mask algebra
                tc.tile_pool(name="pmt", bufs=2) as pmt,        # u8 tight: match temps
                tc.tile_pool(name="pE1", bufs=2) as pE1,        # u8 padded: eqm1
                tc.tile_pool(name="pM8", bufs=2) as pM8,        # u8 padded: match mask
                tc.tile_pool(name="pEE", bufs=1) as pEE,        # u8 padded: emptyE
                tc.tile_pool(name="psw", bufs=2) as psw,        # u8 tight: swaps
            ):
                self.pconst, self.pw32, self.ppack = pconst, pw32, ppack
                self.pmask, self.pf32t, self.pu8t = pmask, pf32t, pu8t
                self.pmt, self.pE1, self.pM8, self.pEE, self.psw = pmt, pE1, pM8, pEE, psw
                if USE_BCAST_CVALS:
                    self.cvals = pconst.tile([P, 9, W], dt.uint8, tag="cvals", name="cvals")
                    for v in range(9):
                        nc.vector.memset(self.cvals[:, v:v + 1, :], v)
                else:
                    self.cvals = pconst.tile([P, 9 * S, W], dt.uint8, tag="cvals", name="cvals")
                    for v in range(9):
                        nc.vector.memset(self.cvals[:, v * S:(v + 1) * S, :], v)
                for b in range(NB):
                    self.image(b)
        nc.compile()
        return nc

    def cval(self, v):
        if USE_BCAST_CVALS:
            return self.cvals[:, v:v + 1, :].to_broadcast([P, S, W])
        return self.cvals[:, v * S:(v + 1) * S, :]

    def u8(self):
        return self.pu8t.tile([P, S, W], dt.uint8, tag="u8t", name="u8t")

    # ---------- halo helpers ----------

    def fill_xcols(self, t, slots=slice(1, 5), engine=None):
        nc = self.nc
        e = engine or nc.vector
        if e is nc.scalar:
            e.copy(out=t[:, slots, 0:1], in_=t[:, slots, W:W + 1])
            e.copy(out=t[:, slots, Wp - 1:Wp], in_=t[:, slots, 1:2])
        else:
            e.tensor_copy(out=t[:, slots, 0:1], in_=t[:, slots, W:W + 1])
            e.tensor_copy(out=t[:, slots, Wp - 1:Wp], in_=t[:, slots, 1:2])

    def fill_xcol_side(self, t, dx, slots=slice(1, 5)):
        nc = self.nc
        if dx > 0:
            nc.vector.tensor_copy(out=t[:, slots, Wp - 1:Wp], in_=t[:, slots, 1:2])
        elif dx < 0:
            nc.vector.tensor_copy(out=t[:, slots, 0:1], in_=t[:, slots, W:W + 1])

    def fill_yhalo(self, t, hi, zero_edge=False):
        # compute ops need 32-aligned partition bases: zero the whole halo slot
        # first, then let the partition-shift DMA overwrite all but the edge row
        nc = self.nc
        if hi:
            if zero_edge:
                nc.vector.memset(t[:, 5, :], 0)
            nc.sync.dma_start(out=t[0:P - 1, 5, :], in_=t[1:P, 1, :])
            if not zero_edge:
                nc.sync.dma_start(out=t[P - 1:P, 5, :], in_=t[0:1, 1, :])
        else:
            if zero_edge:
                nc.vector.memset(t[:, 0, :], 0)
            nc.sync.dma_start(out=t[1:P, 0, :], in_=t[0:P - 1, 4, :])
            if not zero_edge:
                nc.sync.dma_start(out=t[0:1, 0, :], in_=t[P - 1:P, 4, :])

    def fill_halos(self, t):
        self.fill_xcols(t)
        self.fill_yhalo(t, hi=True)
        self.fill_yhalo(t, hi=False)

    # ---------- DRAM loads (iter 0) ----------

    def _load_padded_f32(self, b, c, t):
        nc = self.nc
        d = self.win[b, c].rearrange("(p k) x -> p k x", k=S)  # [128, 4, 512]
        nc.sync.dma_start(out=t[:, 1:1 + S, 1:1 + W], in_=d)
        nc.sync.dma_start(out=t[1:P, 0, 1:1 + W], in_=d[0:P - 1, S - 1, :])
        nc.sync.dma_start(out=t[0:1, 0, 1:1 + W], in_=d[P - 1:P, S - 1, :])
        nc.sync.dma_start(out=t[0:P - 1, 5, 1:1 + W], in_=d[1:P, 0, :])
        nc.sync.dma_start(out=t[P - 1:P, 5, 1:1 + W], in_=d[0:1, 0, :])

    def load_f32_padded(self, b, c):
        t = self.pw32.tile([P, 6, Wp], dt.float32, tag="w32", name="wf32")
        self._load_padded_f32(b, c, t)
        self.fill_xcols(t, slots=slice(0, 6))
        return t

    def load_packed(self, b, pair):
        nc = self.nc
        t = self.ppack.tile([P, 6, Wp], dt.int32, tag="pk", name="pk")
        lo_v, hi_v = _bf16_views(t)
        for ch, view in ((pair[0], lo_v), (pair[1], hi_v)):
            stg = self.pw32.tile([P, 6, Wp], dt.float32, tag="w32", name="stg")
            self._load_padded_f32(b, ch, stg)
            self.fill_xcols(stg, slots=slice(0, 6), engine=nc.scalar)
            nc.scalar.copy(out=view, in_=stg[:])  # cast f32->bf16, strided pack
        return t

    # ---------- phase A: direction masks ----------

    def phase_A(self, w3, w4, pc, thresh_sq):
        """Returns Wm2[a] (u8 tight, = want-move-a & enough & shifted-empty) and emits them."""
        nc = self.nc
        vy = _interior(w3)
        vx = _interior(w4)

        t1 = self.pf32t.tile([P, S, W], dt.float32, tag="f32t", name="t1")
        nc.scalar.activation(t1[:], vy, Act.Square)
        t2 = self.pf32t.tile([P, S, W], dt.float32, tag="f32t", name="t2")
        nc.scalar.activation(t2[:], vx, Act.Square)
        nc.vector.tensor_tensor(out=t1[:], in0=t1[:], in1=t2[:], op=Alu.add)  # m2
        gt = self.u8()
        nc.vector.tensor_scalar(out=gt[:], in0=t1[:], scalar1=thresh_sq, scalar2=None, op0=Alu.is_gt)
        t3 = self.pf32t.tile([P, S, W], dt.float32, tag="f32t", name="t3")
        nc.scalar.activation(t3[:], t1[:], Act.Sqrt)
        nc.vector.scalar_tensor_tensor(out=t3[:], in0=t3[:], scalar=0.002, in1=t1[:],
                                       op0=Alu.mult, op1=Alu.add)             # magp2 - 1e-6
        nc.vector.tensor_scalar(out=t1[:], in0=t3[:], scalar1=1e-6, scalar2=K0SQ,
                                op0=Alu.add, op1=Alu.mult)                    # T0
        g0 = self.u8()
        nc.vector.tensor_tensor(out=g0[:], in0=t2[:], in1=t1[:], op=Alu.is_le)
        nc.vector.tensor_scalar(out=t1[:], in0=t3[:], scalar1=1e-6, scalar2=K1SQ,
                                op0=Alu.add, op1=Alu.mult)                    # T1
        g1 = self.u8()
        nc.vector.tensor_tensor(out=g1[:], in0=t2[:], in1=t1[:], op=Alu.is_le)
        zb = self.u8()
        nc.vector.tensor_scalar(out=zb[:], in0=vx, scalar1=0.0, scalar2=None, op0=Alu.is_le)
        u0 = self.u8()
        nc.vector.tensor_tensor(out=u0[:], in0=zb[:], in1=g0[:], op=Alu.max)
        u1 = self.u8()
        nc.vector.tensor_tensor(out=u1[:], in0=zb[:], in1=g1[:], op=Alu.max)
        # u2 = zb*(1-g1), u3 = zb*(1-g0): reuse g-slots via (g*-1+1)
        nc.vector.tensor_scalar(out=g1[:], in0=g1[:], scalar1=-1.0, scalar2=1.0,
                                op0=Alu.mult, op1=Alu.add)
        u2 = self.u8()
        nc.vector.tensor_tensor(out=u2[:], in0=zb[:], in1=g1[:], op=Alu.mult)
        nc.vector.tensor_scalar(out=g0[:], in0=g0[:], scalar1=-1.0, scalar2=1.0,
                                op0=Alu.mult, op1=Alu.add)
        u3 = self.u8()
        nc.vector.tensor_tensor(out=u3[:], in0=zb[:], in1=g0[:], op=Alu.mult)
        # g0,g1 free; zb free. enough = (m2>th) & (E != 1)
        _, e_hi = _bf16_views(pc)
        wallok = self.u8()
        nc.vector.tensor_scalar(out=wallok[:], in0=e_hi[:, 1:1 + S, 1:1 + W],
                                scalar1=1.0, scalar2=None, op0=Alu.not_equal)
        en = self.u8()
        nc.vector.tensor_tensor(out=en[:], in0=gt[:], in1=wallok[:], op=Alu.mult)
        # gt, wallok free
        s1 = self.u8()
        nc.vector.tensor_scalar(out=s1[:], in0=vy, scalar1=0.0, scalar2=None, op0=Alu.is_lt)
        nc.vector.tensor_tensor(out=s1[:], in0=s1[:], in1=en[:], op=Alu.mult)
        s0 = self.u8()
        nc.vector.tensor_tensor(out=s0[:], in0=en[:], in1=s1[:], op=Alu.subtract)
        # emptyE u8 padded, from E channel (hi bf16 half of pc), all slots incl halos
        emptyE = self.pEE.tile([P, 6, Wp], dt.uint8, tag="EE", name="emptyE")
        nc.vector.tensor_scalar(out=emptyE[:], in0=e_hi, scalar1=0.0, scalar2=None, op0=Alu.is_equal)

        Wm2 = [None] * 8

        def emit_w(a, f0, f1):
            tmp = self.pmt.tile([P, S, W], dt.uint8, tag="mt", name="wtmp")
            nc.vector.tensor_tensor(out=tmp[:], in0=f0[:], in1=f1[:], op=Alu.mult)
            m = self.pmask.tile([P, S, W], dt.uint8, tag="m8", name="wm")
            nc.vector.tensor_tensor(out=m[:], in0=tmp[:], in1=_view(emptyE, _DY[a], _DX[a]), op=Alu.mult)
            Wm2[a] = m

        d1 = self.u8()
        nc.vector.tensor_tensor(out=d1[:], in0=u0[:], in1=u1[:], op=Alu.subtract)
        emit_w(1, d1, s0)
        emit_w(7, d1, s1)
        nc.vector.tensor_scalar(out=u0[:], in0=u0[:], scalar1=-1.0, scalar2=1.0,
                                op0=Alu.mult, op1=Alu.add)   # nu0
        emit_w(0, u0, en)
        d2 = self.u8()
        nc.vector.tensor_tensor(out=d2[:], in0=u1[:], in1=u2[:], op=Alu.subtract)
        emit_w(2, d2, s0)
        emit_w(6, d2, s1)
        d3 = self.u8()
        nc.vector.tensor_tensor(out=d3[:], in0=u2[:], in1=u3[:], op=Alu.subtract)
        emit_w(3, d3, s0)
        emit_w(5, d3, s1)
        emit_w(4, u3, en)
        return Wm2

    # ---------- phase B: sequential swap resolution ----------

    def phase_B(self, Wm2):
        nc = self.nc
        swaps = self.psw.tile([P, S, W], dt.uint8, tag="swaps", name="swaps")
        nc.vector.memset(swaps[:], 8)
        E1 = self.pE1.tile([P, 6, Wp], dt.uint8, tag="E1", name="E1")
        M8 = self.pM8.tile([P, 6, Wp], dt.uint8, tag="M8", name="M8")
        for a in range(8):
            dy, dx = _DY[a], _DX[a]
            a4 = (a + 4) % 8
            dy4, dx4 = -dy, -dx
            if a == 0:
                nc.vector.tensor_copy(out=_interior(M8), in_=Wm2[0][:])
            else:
                nc.vector.tensor_scalar(out=_interior(E1), in0=swaps[:], scalar1=8.0,
                                        scalar2=None, op0=Alu.is_equal)
                self.fill_xcol_side(E1, dx)
                if dy > 0:
                    self.fill_yhalo(E1, hi=True)
                elif dy < 0:
                    self.fill_yhalo(E1, hi=False)
                mtmp = self.pmt.tile([P, S, W], dt.uint8, tag="mt", name="mtmp")
                nc.vector.tensor_tensor(out=mtmp[:], in0=Wm2[a][:], in1=_interior(E1), op=Alu.mult)
                nc.vector.tensor_tensor(out=_interior(M8), in0=mtmp[:], in1=_view(E1, dy, dx), op=Alu.mult)
            self.fill_xcol_side(M8, dx4)
            if dy4 > 0:
                self.fill_yhalo(M8, hi=True)
            elif dy4 < 0:
                self.fill_yhalo(M8, hi=False)
            nc.vector.copy_predicated(out=swaps[:], mask=_interior(M8), data=self.cval(a))
            nc.vector.copy_predicated(out=swaps[:], mask=_view(M8, dy4, dx4), data=self.cval(a4))
        return swaps

    # ---------- phase C: gather ----------

    def phase_C(self, swaps, streams):
        nc = self.nc
        equ8 = []
        for a in range(8):
            m = self.pmask.tile([P, S, W], dt.uint8, tag="m8", name="equ")
            nc.vector.tensor_scalar(out=m[:], in0=swaps[:], scalar1=float(a),
                                    scalar2=None, op0=Alu.is_equal)
            equ8.append(m)
        news = []
        for t, kind in streams:
            if kind == "f32":
                nt = self.pw32.tile([P, 6, Wp], dt.float32, tag="w32", name="nf32")
                nc.scalar.copy(out=_interior(nt), in_=_interior(t))
            else:
                nt = self.ppack.tile([P, 6, Wp], dt.int32, tag="pk", name="npk")
                nc.vector.tensor_copy(out=_interior(nt), in_=_interior(t))
            for a in range(8):
                nc.vector.copy_predicated(out=_interior(nt), mask=equ8[a][:],
                                          data=_view(t, _DY[a], _DX[a]))
            news.append(nt)
        return news

    # ---------- final conv ----------

    def conv_channel(self, vf, out_tight):
        """out = conv3x3(vf, nk) + 0.5*vf (zero padding); vf padded with zeroed edges."""
        nc = self.nc
        nk = self.nk
        uniform = bool(np.allclose(nk, nk[0, 0]))
        kys = [0] if uniform else [0, 1, 2]
        tmps = []
        for ky in kys:
            tp = self.pw32.tile([P, 6, Wp], dt.float32, tag="w32", name="convtp")
            if uniform:
                nc.vector.tensor_tensor(out=_interior(tp), in0=_view(vf, 0, -1),
                                        in1=_view(vf, 0, 0), op=Alu.add)
                nc.vector.tensor_tensor(out=_interior(tp), in0=_interior(tp),
                                        in1=_view(vf, 0, 1), op=Alu.add)
            else:
                nc.scalar.mul(_interior(tp), _view(vf, 0, 0), float(nk[ky, 1]))
                nc.vector.scalar_tensor_tensor(out=_interior(tp), in0=_view(vf, 0, -1),
                                               scalar=float(nk[ky, 0]), in1=_interior(tp),
                                               op0=Alu.mult, op1=Alu.add)
                nc.vector.scalar_tensor_tensor(out=_interior(tp), in0=_view(vf, 0, 1),
                                               scalar=float(nk[ky, 2]), in1=_interior(tp),
                                               op0=Alu.mult, op1=Alu.add)
            self.fill_yhalo(tp, hi=True, zero_edge=True)
            self.fill_yhalo(tp, hi=False, zero_edge=True)
            tmps.append(tp)
        if uniform:
            tmps = [tmps[0]] * 3
        acc = self.pf32t.tile([P, S, W], dt.float32, tag="f32t", name="acc")
        nc.vector.tensor_tensor(out=acc[:], in0=_view(tmps[0], -1, 0),
                                in1=_view(tmps[1], 0, 0), op=Alu.add)
        nc.vector.tensor_tensor(out=acc[:], in0=acc[:], in1=_view(tmps[2], 1, 0), op=Alu.add)
        vfh = self.pf32t.tile([P, S, W], dt.float32, tag="f32t", name="vfh")
        nc.scalar.mul(vfh[:], _interior(vf), 0.5)
        scale = float(nk[0, 0]) if uniform else 1.0
        nc.vector.scalar_tensor_tensor(out=out_tight[:], in0=acc[:], scalar=scale,
                                       in1=vfh[:], op0=Alu.mult, op1=Alu.add)

    # ---------- per-image program ----------

    def image(self, b):
        nc = self.nc
        w3 = self.load_f32_padded(b, 3)
        w4 = self.load_f32_padded(b, 4)
        packs = [self.load_packed(b, pr) for pr in PACKS]

        for n in range(2):
            thresh_sq = 1.0 if n == 0 else 4.0
            Wm2 = self.phase_A(w3, w4, packs[2], thresh_sq)
            swaps = self.phase_B(Wm2)
            streams = [(packs[0], "pack"), (packs[1], "pack"), (packs[2], "pack"),
                       (w3, "f32"), (w4, "f32")]
            npa, npb, npc, n3, n4 = self.phase_C(swaps, streams)
            for old, new in ((w3, n3), (w4, n4)):
                vh = self.pf32t.tile([P, S, W], dt.float32, tag="f32t", name="vh")
                nc.scalar.mul(vh[:], _interior(old), 0.5)
                nc.vector.scalar_tensor_tensor(out=_interior(new), in0=_interior(new),
                                               scalar=0.5, in1=vh[:], op0=Alu.mult, op1=Alu.add)
            packs = [npa, npb, npc]
            w3, w4 = n3, n4
            if n == 0:
                for t in packs + [w3, w4]:
                    self.fill_halos(t)

        # final: vel *= 0.95, zero-padded halos, 3x3 smoothing conv
        for c, vf in ((3, w3), (4, w4)):
            nc.scalar.mul(_interior(vf), _interior(vf), 0.95)
            nc.vector.memset(vf[:, 1:5, 0:1], 0)
            nc.vector.memset(vf[:, 1:5, Wp - 1:Wp], 0)
            self.fill_yhalo(vf, hi=True, zero_edge=True)
            self.fill_yhalo(vf, hi=False, zero_edge=True)
            ot = self.pf32t.tile([P, S, W], dt.float32, tag="f32t", name="convout")
            self.conv_channel(vf, ot)
            nc.sync.dma_start(out=self.wout[b, c].rearrange("(p k) x -> p k x", k=S), in_=ot[:])

        for (clo, chi), t in zip(PACKS, packs):
            lo_v, hi_v = _bf16_views(t)
            for ch, view in ((clo, lo_v), (chi, hi_v)):
                stg = self.pf32t.tile([P, S, W], dt.float32, tag="f32t", name="ostg")
                nc.scalar.copy(out=stg[:], in_=view[:, 1:1 + S, 1:1 + W])
                nc.sync.dma_start(out=self.wout[b, ch].rearrange("(p k) x -> p k x", k=S),
                                  in_=stg[:])


def _build(nk):
    return _Emit(nk).build()


def kernel(world, rand_movement=None, rand_interact=None, rand_element=None,
           neighbor_kernel=None, **_kw):
    world = np.ascontiguousarray(np.asarray(world, dtype=np.float32))
    nk = np.asarray(neighbor_kernel, dtype=np.float32).reshape(3, 3) / 18.0
    key = nk.tobytes()
    nc = _cache.get(key)
    if nc is None:
        nc = _cache[key] = _build(nk)
    in_maps = [{"w": world[NB * i:NB * (i + 1)]} for i in range(NCORES)]
    res = run_bass_kernel_spmd(nc, in_maps, list(range(NCORES))).results
    return np.concatenate([r["o"] for r in res], axis=0)


# revision 36
# speedup vs baseline: 4.3878x; 4.3878x over previous
"""Trainium2 Bass kernel for nn_BehaviorVelocity (velocity-driven swap sim + smoothing).

Sharding: data-parallel over batch B=16 across 8 cores (2 images/core, no collectives).

Layout per 512x512 image: partition p holds rows 4p..4p+3 as free-dim "slots".
Padded field = [128, 6 slots, 514 cols]:
  slot 0 = row 4p-1 (y-halo lo), slots 1..4 = rows 4p..4p+3, slot 5 = row 4p+4 (y-halo hi)
  col 0 = x=511 (wrap), cols 1..512 = x=0..511, col 513 = x=0 (wrap)
y-halos: partition-shift SBUF DMAs (+1-row torus wrap DMA). x-halos: tiny strided copies.
All spatial shifts then become free-dim AP offsets (compute ops must start at partition 0).

Channels 3,4 (vy,vx) stay f32. Channels (1,2),(5,6),(7,0) are packed as bf16 pairs
inside int32 containers so each (mode-less, 1x) copy_predicated moves 2 channels.

Sector selection replicates floor(8*arccos-angle+0.5) via threshold compares in the
squared domain:  vx <= K*(mag+0.001)  <=>  (vx<=0) or (vx^2 <= K^2*magp2)  with
magp2 = m2 + 0.002*mag + 1e-6, so the ACT-sqrt LUT error only enters the tiny
0.002*mag term (~5e-8 boundary shift instead of ~3e-5).
"""

import sys

sys.path.insert(0, "/opt/trn_rl_repo")

import numpy as np

import concourse.bacc as bacc
import concourse.mybir as mybir
from concourse.tile import TileContext
from concourse.bass_utils import run_bass_kernel_spmd

dt = mybir.dt
Alu = mybir.AluOpType
Act = mybir.ActivationFunctionType

P = 128          # partitions
S = 4            # row-slots per partition (512 rows / 128)
W = 512
Wp = W + 2       # 514 with x-halo cols
NB = 2           # batch images per core
NCORES = 8

_DY = [0, 1, 1, 1, 0, -1, -1, -1]
_DX = [1, 1, 0, -1, -1, -1, 0, 1]

K0SQ = float(np.cos(np.pi / 8) ** 2)      # 0.85355339059
K1SQ = float(np.cos(3 * np.pi / 8) ** 2)  # 0.14644660941

PACKS = [(1, 2), (5, 6), (7, 0)]  # (lo, hi) bf16 pair in int32; E-channel 0 = hi of last

USE_BCAST_CVALS = True
REPEAT = 1  # profiling knob: emit the whole pipeline N times

_cache = {}


def _interior(t):
    return t[:, 1:1 + S, 1:1 + W]


def _view(t, dy, dx):
    # value of neighbor at (y+dy, x+dx) for each interior pixel
    return t[:, 1 + dy:1 + S + dy, 1 + dx:1 + W + dx]


def _bf16_views(t_i32):
    """(lo, hi) bf16 strided views [P,6,Wp] of an int32 [P,6,Wp] padded tile."""
    b = t_i32[:].bitcast(dt.bfloat16)            # [P, 6, 2*Wp]
    b = b.rearrange("p s (c two) -> p s c two", two=2)
    return b[:, :, :, 0], b[:, :, :, 1]


class _Emit:
    def __init__(self, nk):
        self.nk = nk  # 3x3 conv kernel (already /18)
        nc = self.nc = bacc.Bacc()
        self.win = nc.declare_dram_parameter("w", [NB, 8, 512, 512], dt.float32, isOutput=False)
        self.wout = nc.declare_dram_parameter("o", [NB, 8, 512, 512], dt.float32, isOutput=True)

    def build(self):
        nc = self.nc
        with TileContext(nc) as tc:
            self.tc = tc
            with (
                tc.tile_pool(name="pconst", bufs=1) as pconst,
                tc.tile_pool(name="pw32", bufs=5) as pw32,      # f32 padded [P,6,Wp]
                tc.tile_pool(name="ppack", bufs=4) as ppack,    # int32 padded [P,6,Wp]
                tc.tile_pool(name="pmask", bufs=12) as pmask,    # u8 tight [P,S,W]: equ8
                tc.tile_pool(name="pf32t", bufs=4) as pf32t,    # f32 tight [P,S,W]
                tc.tile_pool(name="pu8t", bufs=12) as pu8t,     # u8 tight: mask algebra + Wm2
                tc.tile_pool(name="pE1", bufs=2) as pE1,        # bf16 padded: eqm1
                tc.tile_pool(name="pM8", bufs=2) as pM8,        # u8 padded: match mask
                tc.tile_pool(name="pEE", bufs=1) as pEE,        # bf16 padded: emptyE
                tc.tile_pool(name="psw", bufs=2) as psw,        # u8 tight: swaps
            ):
                self.pconst, self.pw32, self.ppack = pconst, pw32, ppack
                self.pmask, self.pf32t, self.pu8t = pmask, pf32t, pu8t
                self.pE1, self.pM8, self.pEE, self.psw = pE1, pM8, pEE, psw
                if USE_BCAST_CVALS:
                    self.cvals = pconst.tile([P, 9, 4], dt.uint8, tag="cvals", name="cvals")
                    for v in range(9):
                        nc.vector.memset(self.cvals[:, v:v + 1, :], v)
                else:
                    self.cvals = pconst.tile([P, 9 * S, W], dt.uint8, tag="cvals", name="cvals")
                    for v in range(9):
                        nc.vector.memset(self.cvals[:, v * S:(v + 1) * S, :], v)
                for _r in range(REPEAT):
                    for b in range(NB):
                        st = self.image_load(b)
                        for n in range(2):
                            self.image_iter(st, n)
                        self.image_final(b, st)
        nc.compile()
        return nc

    def cval(self, v):
        if USE_BCAST_CVALS:
            return self.cvals[:, v:v + 1, 0:1].to_broadcast([P, S, W])
        return self.cvals[:, v * S:(v + 1) * S, :]

    def u8(self):
        return self.pu8t.tile([P, S, W], dt.uint8, tag="bft", name="bft")

    # ---------- halo helpers ----------

    def fill_xcols(self, t, slots=slice(1, 5), engine=None):
        nc = self.nc
        e = engine or nc.vector
        if e is nc.scalar:
            e.copy(out=t[:, slots, 0:1], in_=t[:, slots, W:W + 1])
            e.copy(out=t[:, slots, Wp - 1:Wp], in_=t[:, slots, 1:2])
        else:
            e.tensor_copy(out=t[:, slots, 0:1], in_=t[:, slots, W:W + 1])
            e.tensor_copy(out=t[:, slots, Wp - 1:Wp], in_=t[:, slots, 1:2])

    def fill_xcol_side(self, t, dx, slots=slice(1, 5)):
        nc = self.nc
        if dx > 0:
            nc.vector.tensor_copy(out=t[:, slots, Wp - 1:Wp], in_=t[:, slots, 1:2])
        elif dx < 0:
            nc.vector.tensor_copy(out=t[:, slots, 0:1], in_=t[:, slots, W:W + 1])

    def fill_yhalo(self, t, hi, zero_edge=False, dma=None):
        # compute ops need 32-aligned partition bases: zero the whole halo slot
        # first, then let the partition-shift DMA overwrite all but the edge row
        nc = self.nc
        dma = dma or nc.scalar
        if hi:
            if zero_edge:
                nc.vector.memset(t[:, 5, :], 0)
            dma.dma_start(out=t[0:P - 1, 5, :], in_=t[1:P, 1, :])
            if not zero_edge:
                dma.dma_start(out=t[P - 1:P, 5, :], in_=t[0:1, 1, :])
        else:
            if zero_edge:
                nc.vector.memset(t[:, 0, :], 0)
            dma.dma_start(out=t[1:P, 0, :], in_=t[0:P - 1, 4, :])
            if not zero_edge:
                dma.dma_start(out=t[0:1, 0, :], in_=t[P - 1:P, 4, :])

    def fill_halos(self, t):
        self.fill_xcols(t)
        self.fill_yhalo(t, hi=True)
        self.fill_yhalo(t, hi=False)

    # ---------- DRAM loads (iter 0) ----------

    def _load_padded_f32(self, b, c, t):
        nc = self.nc
        d = self.win[b, c].rearrange("(p k) x -> p k x", k=S)  # [128, 4, 512]
        nc.sync.dma_start(out=t[:, 1:1 + S, 1:1 + W], in_=d)
        nc.sync.dma_start(out=t[1:P, 0, 1:1 + W], in_=d[0:P - 1, S - 1, :])
        nc.sync.dma_start(out=t[0:1, 0, 1:1 + W], in_=d[P - 1:P, S - 1, :])
        nc.sync.dma_start(out=t[0:P - 1, 5, 1:1 + W], in_=d[1:P, 0, :])
        nc.sync.dma_start(out=t[P - 1:P, 5, 1:1 + W], in_=d[0:1, 0, :])

    def load_f32_padded(self, b, c):
        t = self.pw32.tile([P, 6, Wp], dt.float32, tag="w32", name="wf32")
        self._load_padded_f32(b, c, t)
        self.fill_xcols(t, slots=slice(0, 6))
        return t

    def load_packed(self, b, pair):
        nc = self.nc
        t = self.ppack.tile([P, 6, Wp], dt.int32, tag="pk", name="pk")
        lo_v, hi_v = _bf16_views(t)
        for ch, view in ((pair[0], lo_v), (pair[1], hi_v)):
            stg = self.pw32.tile([P, 6, Wp], dt.float32, tag="w32", name="stg")
            self._load_padded_f32(b, ch, stg)
            self.fill_xcols(stg, slots=slice(0, 6), engine=nc.scalar)
            nc.scalar.copy(out=view, in_=stg[:])  # cast f32->bf16, strided pack
        return t

    # ---------- phase A: direction masks ----------

    def phase_A(self, w3, w4, pc, thresh_sq):
        """Returns Wm2[a] (u8 tight, = want-move-a & enough & shifted-empty) and emits them."""
        nc = self.nc
        vy = _interior(w3)
        vx = _interior(w4)

        t1 = self.pf32t.tile([P, S, W], dt.float32, tag="f32t", name="t1")
        nc.scalar.activation(t1[:], vy, Act.Square)
        t2 = self.pf32t.tile([P, S, W], dt.float32, tag="f32t", name="t2")
        nc.scalar.activation(t2[:], vx, Act.Square)
        nc.vector.tensor_tensor(out=t1[:], in0=t1[:], in1=t2[:], op=Alu.add)  # m2
        gt = self.u8()
        nc.vector.tensor_scalar(out=gt[:], in0=t1[:], scalar1=thresh_sq, scalar2=None, op0=Alu.is_gt)
        t3 = self.pf32t.tile([P, S, W], dt.float32, tag="f32t", name="t3")
        nc.scalar.activation(t3[:], t1[:], Act.Sqrt)
        nc.vector.scalar_tensor_tensor(out=t3[:], in0=t3[:], scalar=0.002, in1=t1[:],
                                       op0=Alu.mult, op1=Alu.add)             # magp2 - 1e-6
        nc.scalar.activation(t1[:], t3[:], Act.Copy, bias=1e-6 * K0SQ, scale=K0SQ)  # T0
        g0 = self.u8()
        nc.vector.tensor_tensor(out=g0[:], in0=t2[:], in1=t1[:], op=Alu.is_le)
        nc.scalar.activation(t1[:], t3[:], Act.Copy, bias=1e-6 * K1SQ, scale=K1SQ)  # T1
        g1 = self.u8()
        nc.vector.tensor_tensor(out=g1[:], in0=t2[:], in1=t1[:], op=Alu.is_le)
        zb = self.u8()
        nc.vector.tensor_scalar(out=zb[:], in0=vx, scalar1=0.0, scalar2=None, op0=Alu.is_le)
        u0 = self.u8()
        nc.vector.tensor_tensor(out=u0[:], in0=zb[:], in1=g0[:], op=Alu.max)
        u1 = self.u8()
        nc.vector.tensor_tensor(out=u1[:], in0=zb[:], in1=g1[:], op=Alu.max)
        # u2 = zb*(1-g1), u3 = zb*(1-g0): reuse g-slots via (g*-1+1)
        nc.vector.tensor_scalar(out=g1[:], in0=g1[:], scalar1=-1.0, scalar2=1.0,
                                op0=Alu.mult, op1=Alu.add)
        u2 = self.u8()
        nc.vector.tensor_tensor(out=u2[:], in0=zb[:], in1=g1[:], op=Alu.mult)
        nc.vector.tensor_scalar(out=g0[:], in0=g0[:], scalar1=-1.0, scalar2=1.0,
                                op0=Alu.mult, op1=Alu.add)
        u3 = self.u8()
        nc.vector.tensor_tensor(out=u3[:], in0=zb[:], in1=g0[:], op=Alu.mult)
        # g0,g1 free; zb free. enough = (m2>th) & (E != 1)
        _, e_hi = _bf16_views(pc)
        wallok = self.u8()
        nc.vector.tensor_scalar(out=wallok[:], in0=e_hi[:, 1:1 + S, 1:1 + W],
                                scalar1=1.0, scalar2=None, op0=Alu.not_equal)
        en = self.u8()
        nc.vector.tensor_tensor(out=en[:], in0=gt[:], in1=wallok[:], op=Alu.mult)
        # gt, wallok free
        s1 = self.u8()
        nc.vector.tensor_scalar(out=s1[:], in0=vy, scalar1=0.0, scalar2=None, op0=Alu.is_lt)
        nc.vector.tensor_tensor(out=s1[:], in0=s1[:], in1=en[:], op=Alu.mult)
        s0 = self.u8()
        nc.vector.tensor_tensor(out=s0[:], in0=en[:], in1=s1[:], op=Alu.subtract)
        # emptyE u8 padded, from E channel (hi bf16 half of pc), all slots incl halos
        emptyE = self.pEE.tile([P, 6, Wp], dt.uint8, tag="EE", name="emptyE")
        nc.vector.tensor_scalar(out=emptyE[:], in0=e_hi, scalar1=0.0, scalar2=None, op0=Alu.is_equal)

        Wm2 = [None] * 8

        def emit_w(a, f0, f1):
            eng = nc.vector
            tmp = self.pu8t.tile([P, S, W], dt.uint8, tag="bft", name="wtmp")
            eng.tensor_tensor(out=tmp[:], in0=f0[:], in1=f1[:], op=Alu.mult)
            m = self.pu8t.tile([P, S, W], dt.uint8, tag="bft", name="wm")
            eng.tensor_tensor(out=m[:], in0=tmp[:], in1=_view(emptyE, _DY[a], _DX[a]), op=Alu.mult)
            Wm2[a] = m

        d1 = self.u8()
        nc.vector.tensor_tensor(out=d1[:], in0=u0[:], in1=u1[:], op=Alu.subtract)
        emit_w(1, d1, s0)
        emit_w(7, d1, s1)
        nc.vector.tensor_scalar(out=u0[:], in0=u0[:], scalar1=-1.0, scalar2=1.0,
                                op0=Alu.mult, op1=Alu.add)   # nu0
        emit_w(0, u0, en)
        d2 = self.u8()
        nc.vector.tensor_tensor(out=d2[:], in0=u1[:], in1=u2[:], op=Alu.subtract)
        emit_w(2, d2, s0)
        emit_w(6, d2, s1)
        d3 = self.u8()
        nc.vector.tensor_tensor(out=d3[:], in0=u2[:], in1=u3[:], op=Alu.subtract)
        emit_w(3, d3, s0)
        emit_w(5, d3, s1)
        emit_w(4, u3, en)
        return Wm2

    # ---------- phase B: sequential swap resolution ----------

    def phase_B(self, Wm2):
        nc = self.nc
        swaps = self.psw.tile([P, S, W], dt.uint8, tag="swaps", name="swaps")
        nc.vector.memset(swaps[:], 8)
        E1 = self.pE1.tile([P, 6, Wp], dt.uint8, tag="E1", name="E1")
        M8 = self.pM8.tile([P, 6, Wp], dt.uint8, tag="M8", name="M8")
        for a in range(8):
            dy, dx = _DY[a], _DX[a]
            a4 = (a + 4) % 8
            dy4, dx4 = -dy, -dx
            if a == 0:
                nc.vector.tensor_scalar(out=_interior(M8), in0=Wm2[0][:], scalar1=0.0,
                                        scalar2=None, op0=Alu.is_gt)
            else:
                nc.vector.tensor_scalar(out=_interior(E1), in0=swaps[:], scalar1=8.0,
                                        scalar2=None, op0=Alu.is_equal)
                self.fill_xcol_side(E1, dx)
                if dy > 0:
                    self.fill_yhalo(E1, hi=True)
                elif dy < 0:
                    self.fill_yhalo(E1, hi=False)
                mtmp = self.pu8t.tile([P, S, W], dt.uint8, tag="bft", name="mtmp")
                nc.vector.tensor_tensor(out=mtmp[:], in0=Wm2[a][:], in1=_interior(E1), op=Alu.mult)
                nc.vector.tensor_tensor(out=_interior(M8), in0=mtmp[:], in1=_view(E1, dy, dx), op=Alu.mult)
            self.fill_xcol_side(M8, dx4)
            if dy4 > 0:
                self.fill_yhalo(M8, hi=True)
            elif dy4 < 0:
                self.fill_yhalo(M8, hi=False)
            nc.vector.copy_predicated(out=swaps[:], mask=_interior(M8), data=self.cval(a))
            nc.vector.copy_predicated(out=swaps[:], mask=_view(M8, dy4, dx4), data=self.cval(a4))
        return swaps

    # ---------- phase C: gather ----------

    def phase_C(self, swaps, streams):
        nc = self.nc
        equ8 = []
        for a in range(8):
            m = self.pmask.tile([P, S, W], dt.uint8, tag="m8", name="equ")
            nc.vector.tensor_scalar(out=m[:], in0=swaps[:], scalar1=float(a),
                                    scalar2=None, op0=Alu.is_equal)
            equ8.append(m)
        news = []
        for t, kind in streams:
            if kind == "f32":
                nt = self.pw32.tile([P, 6, Wp], dt.float32, tag="w32", name="nf32")
                nc.scalar.copy(out=_interior(nt), in_=_interior(t))
            else:
                nt = self.ppack.tile([P, 6, Wp], dt.int32, tag="pk", name="npk")
                nc.sync.dma_start(out=_interior(nt), in_=_interior(t))
            for a in range(8):
                nc.vector.copy_predicated(out=_interior(nt), mask=equ8[a][:],
                                          data=_view(t, _DY[a], _DX[a]))
            news.append(nt)
        return news

    # ---------- final conv ----------

    def conv_channel(self, vf, out_tight):
        """out = conv3x3(vf, nk) + 0.5*vf (zero padding); vf padded with zeroed edges."""
        nc = self.nc
        nk = self.nk
        uniform = bool(np.allclose(nk, nk[0, 0]))
        kys = [0] if uniform else [0, 1, 2]
        tmps = []
        for ky in kys:
            tp = self.pw32.tile([P, 6, Wp], dt.float32, tag="w32", name="convtp")
            if uniform:
                nc.vector.tensor_tensor(out=_interior(tp), in0=_view(vf, 0, -1),
                                        in1=_view(vf, 0, 0), op=Alu.add)
                nc.vector.tensor_tensor(out=_interior(tp), in0=_interior(tp),
                                        in1=_view(vf, 0, 1), op=Alu.add)
            else:
                nc.scalar.mul(_interior(tp), _view(vf, 0, 0), float(nk[ky, 1]))
                nc.vector.scalar_tensor_tensor(out=_interior(tp), in0=_view(vf, 0, -1),
                                               scalar=float(nk[ky, 0]), in1=_interior(tp),
                                               op0=Alu.mult, op1=Alu.add)
                nc.vector.scalar_tensor_tensor(out=_interior(tp), in0=_view(vf, 0, 1),
                                               scalar=float(nk[ky, 2]), in1=_interior(tp),
                                               op0=Alu.mult, op1=Alu.add)
            self.fill_yhalo(tp, hi=True, zero_edge=True)
            self.fill_yhalo(tp, hi=False, zero_edge=True)
            tmps.append(tp)
        if uniform:
            tmps = [tmps[0]] * 3
        acc = self.pf32t.tile([P, S, W], dt.float32, tag="f32t", name="acc")
        nc.vector.tensor_tensor(out=acc[:], in0=_view(tmps[0], -1, 0),
                                in1=_view(tmps[1], 0, 0), op=Alu.add)
        nc.vector.tensor_tensor(out=acc[:], in0=acc[:], in1=_view(tmps[2], 1, 0), op=Alu.add)
        vfh = self.pf32t.tile([P, S, W], dt.float32, tag="f32t", name="vfh")
        nc.scalar.mul(vfh[:], _interior(vf), 0.5)
        scale = float(nk[0, 0]) if uniform else 1.0
        nc.vector.scalar_tensor_tensor(out=out_tight[:], in0=acc[:], scalar=scale,
                                       in1=vfh[:], op0=Alu.mult, op1=Alu.add)

    # ---------- per-image program ----------

    def image_load(self, b):
        st = {}
        st["w3"] = self.load_f32_padded(b, 3)
        st["w4"] = self.load_f32_padded(b, 4)
        st["packs"] = [self.load_packed(b, pr) for pr in PACKS]
        return st

    def image_iter(self, st, n):
        nc = self.nc
        w3, w4, packs = st["w3"], st["w4"], st["packs"]
        thresh_sq = 1.0 if n == 0 else 4.0
        Wm2 = self.phase_A(w3, w4, packs[2], thresh_sq)
        swaps = self.phase_B(Wm2)
        streams = [(packs[0], "pack"), (packs[1], "pack"), (packs[2], "pack"),
                   (w3, "f32"), (w4, "f32")]
        npa, npb, npc, n3, n4 = self.phase_C(swaps, streams)
        for old, new in ((w3, n3), (w4, n4)):
            vh = self.pf32t.tile([P, S, W], dt.float32, tag="f32t", name="vh")
            nc.scalar.mul(vh[:], _interior(old), 0.5)
            nc.vector.scalar_tensor_tensor(out=_interior(new), in0=_interior(new),
                                           scalar=0.5, in1=vh[:], op0=Alu.mult, op1=Alu.add)
        st["packs"] = [npa, npb, npc]
        st["w3"], st["w4"] = n3, n4
        if n == 0:
            for t in st["packs"] + [n3, n4]:
                self.fill_halos(t)

    def image_final(self, b, st):
        nc = self.nc
        w3, w4, packs = st["w3"], st["w4"], st["packs"]

        # final: vel *= 0.95, zero-padded halos, 3x3 smoothing conv
        for c, vf in ((3, w3), (4, w4)):
            nc.scalar.mul(_interior(vf), _interior(vf), 0.95)
            nc.vector.memset(vf[:, 1:5, 0:1], 0)
            nc.vector.memset(vf[:, 1:5, Wp - 1:Wp], 0)
            self.fill_yhalo(vf, hi=True, zero_edge=True)
            self.fill_yhalo(vf, hi=False, zero_edge=True)
            ot = self.pf32t.tile([P, S, W], dt.float32, tag="f32t", name="convout")
            self.conv_channel(vf, ot)
            nc.sync.dma_start(out=self.wout[b, c].rearrange("(p k) x -> p k x", k=S), in_=ot[:])

        for (clo, chi), t in zip(PACKS, packs):
            lo_v, hi_v = _bf16_views(t)
            for ch, view in ((clo, lo_v), (chi, hi_v)):
                stg = self.pf32t.tile([P, S, W], dt.float32, tag="f32t", name="ostg")
                nc.scalar.copy(out=stg[:], in_=view[:, 1:1 + S, 1:1 + W])
                nc.sync.dma_start(out=self.wout[b, ch].rearrange("(p k) x -> p k x", k=S),
                                  in_=stg[:])


def _build(nk):
    return _Emit(nk).build()


def kernel(world, rand_movement=None, rand_interact=None, rand_element=None,
           neighbor_kernel=None, **_kw):
    world = np.ascontiguousarray(np.asarray(world, dtype=np.float32))
    nk = np.asarray(neighbor_kernel, dtype=np.float32).reshape(3, 3) / 18.0
    key = nk.tobytes()
    nc = _cache.get(key)
    if nc is None:
        nc = _cache[key] = _build(nk)
    in_maps = [{"w": world[NB * i:NB * (i + 1)]} for i in range(NCORES)]
    res = run_bass_kernel_spmd(nc, in_maps, list(range(NCORES))).results
    return np.concatenate([r["o"] for r in res], axis=0)
